# revision 7
# baseline (speedup 1.0000x reference)
"""GRU decoder kernel for Trainium2 (8 NeuronCores, data-parallel over batch).

Math (PyTorch GRU, gate order r,z,n), per batch element:
    gx_t = x_t * w_ih + b_ih              (input dim == 1 -> rank-1)
    gh_t = h_{t-1} @ w_hh.T + b_hh
    r = sigmoid(gx_r + gh_r); z = sigmoid(gx_z + gh_z)
    n = tanh(gx_n + b_ih_n + r * (gh_n + b_hh_n))
    h_t = (1-z)*n + z*h_{t-1}
    out = h_T @ fc_w.T + fc_b

Device layout (per core, B_c = 1024 batch):
  - partition-stacked: batch 0-511 ("u") on SBUF partitions 0-63,
    batch 512-1023 ("v") on partitions 64-127.  All elementwise ops are
    [128, 512] (gate dim j on partitions within each half, batch on free dim).
  - Two concurrent PE chains via tile_position row/col quadrants:
    u-chain rows 0-1 cols 0-1, v-chain rows 2-3 cols 2-3.
  - x contribution per step via a one-hot K=64 matmul: lhsT[k,m] =
    (k==q)*w_ih[m], rhs = block of 64 timesteps of x -> selects timestep q.
  - biases folded into activation bias (per-partition) and the
    scalar_tensor_tensor per-partition scalar; no bias matmuls.
  - fp16 SBUF tensors, fp32 PSUM accumulation.
"""

import os
import sys

sys.path.insert(0, "/opt/trn_rl_repo")

import numpy as np
from contextlib import ExitStack

HIDDEN = 64
OUT = 256
B = 8192
T = int(os.environ.get("GRU_T", 1024))
NCORES = 8
BC = B // NCORES          # 1024 batch per core
HB = BC // 2              # 512 batch per partition-half
UNROLL = 64               # steps per loop body (one-hot q index is static)
NGROUP = int(os.environ.get("GRU_NGROUP", 2))  # phase-shifted batch groups per core
NBLK = T // UNROLL        # number of 64-step blocks

_CACHE = {}


def _build():
    import concourse.bass as bass
    import concourse.tile as tile
    from concourse import bacc, mybir

    f16 = mybir.dt.float16
    f32 = mybir.dt.float32
    AF = mybir.ActivationFunctionType
    OP = mybir.AluOpType

    nc = bacc.Bacc("TRN2", target_bir_lowering=False, debug=False,
                   num_devices=NCORES)

    d_x = nc.dram_tensor("xt", [128, NBLK, HB], f16, kind="ExternalInput").ap()
    d_wr = nc.dram_tensor("wr", [128, 64], f16, kind="ExternalInput").ap()
    d_wz = nc.dram_tensor("wz", [128, 64], f16, kind="ExternalInput").ap()
    d_wn = nc.dram_tensor("wn", [128, 64], f16, kind="ExternalInput").ap()
    d_ohr = nc.dram_tensor("ohr", [128, UNROLL, 64], f16, kind="ExternalInput").ap()
    d_ohz = nc.dram_tensor("ohz", [128, UNROLL, 64], f16, kind="ExternalInput").ap()
    d_ohn = nc.dram_tensor("ohn", [128, UNROLL, 64], f16, kind="ExternalInput").ap()
    d_br = nc.dram_tensor("br", [128, 1], f32, kind="ExternalInput").ap()
    d_bz = nc.dram_tensor("bz", [128, 1], f32, kind="ExternalInput").ap()
    d_bnh = nc.dram_tensor("bnh", [128, 1], f32, kind="ExternalInput").ap()
    d_bni = nc.dram_tensor("bni", [128, 1], f32, kind="ExternalInput").ap()
    d_fcw = nc.dram_tensor("fcw", [128, OUT], f16, kind="ExternalInput").ap()
    d_fcb = nc.dram_tensor("fcb", [128, 2], f32, kind="ExternalInput").ap()
    d_out = nc.dram_tensor("out", [OUT, BC], f32, kind="ExternalOutput").ap()

    with tile.TileContext(nc) as tc, ExitStack() as ctx:
        singles = ctx.enter_context(tc.tile_pool(name="singles", bufs=1))
        work = ctx.enter_context(tc.tile_pool(name="work", bufs=3))
        psum = ctx.enter_context(tc.tile_pool(name="psum", bufs=1, space="PSUM"))

        X = singles.tile([128, NBLK, HB], f16)
        WR = singles.tile([128, 64], f16)
        WZ = singles.tile([128, 64], f16)
        WN = singles.tile([128, 64], f16)
        OHR = singles.tile([128, UNROLL, 64], f16)
        OHZ = singles.tile([128, UNROLL, 64], f16)
        OHN = singles.tile([128, UNROLL, 64], f16)
        BR = singles.tile([128, 1], f32)
        BZ = singles.tile([128, 1], f32)
        BNH = singles.tile([128, 1], f32)
        BNI = singles.tile([128, 1], f32)
        FCW = singles.tile([128, OUT], f16)
        FCB = singles.tile([128, 2], f32)
        H = singles.tile([128, HB], f16)

        for dst, src in ((X, d_x), (WR, d_wr), (WZ, d_wz), (WN, d_wn),
                         (OHR, d_ohr), (OHZ, d_ohz), (OHN, d_ohn),
                         (BR, d_br), (BZ, d_bz), (BNH, d_bnh), (BNI, d_bni),
                         (FCW, d_fcw), (FCB, d_fcb)):
            nc.gpsimd.dma_start(dst[:], src[:])
        nc.vector.memset(H[:], 0.0)

        HG = HB // NGROUP   # free-dim width per pipelined batch group

        def step(q, xsb, g):
            fd = slice(g * HG, (g + 1) * HG)
            bankR = psum.tile([128, HG], f32, tag=f"bankR{g}")
            bankZ = psum.tile([128, HG], f32, tag=f"bankZ{g}")
            bankN = psum.tile([128, HG], f32, tag=f"bankN{g}")
            bankX = psum.tile([128, HG], f32, tag=f"bankX{g}")
            # x rows live on the OPPOSITE partition half (X-swap) so the
            # one-hot x-matmuls use the other PE row-group: all four
            # quadrant chains (u-h, v-h, u-x, v-x) run concurrently.
            for lo, xlo in ((0, 0), (64, 64)):
                sl = slice(lo, lo + 64)
                xsl = slice(xlo, xlo + 64)
                hs = H[sl, fd]
                xs = xsb[xsl, :, fd]
                tp = (lo, lo)
                xtp = (xlo, lo)
                nc.tensor.matmul(bankR[sl, :], WR[sl, :], hs,
                                 start=True, stop=False, tile_position=tp)
                nc.tensor.matmul(bankR[sl, :], OHR[xsl, q, :], xs,
                                 start=False, stop=True, tile_position=xtp)
                nc.tensor.matmul(bankZ[sl, :], WZ[sl, :], hs,
                                 start=True, stop=False, tile_position=tp)
                nc.tensor.matmul(bankZ[sl, :], OHZ[xsl, q, :], xs,
                                 start=False, stop=True, tile_position=xtp)
                nc.tensor.matmul(bankN[sl, :], WN[sl, :], hs,
                                 start=True, stop=True, tile_position=tp)
                nc.tensor.matmul(bankX[sl, :], OHN[xsl, q, :], xs,
                                 start=True, stop=True, tile_position=xtp)
            SR = work.tile([128, HG], f16, tag=f"SR{g}")
            SZ = work.tile([128, HG], f16, tag=f"SZ{g}")
            T1 = work.tile([128, HG], f16, tag=f"T1{g}")
            T2 = work.tile([128, HG], f16, tag=f"T2{g}")
            NN = work.tile([128, HG], f16, tag=f"NN{g}")
            U = work.tile([128, HG], f16, tag=f"U{g}")
            V = work.tile([128, HG], f16, tag=f"V{g}")
            nc.scalar.activation(SR[:], bankR[:], AF.Sigmoid, bias=BR[:])
            nc.scalar.activation(SZ[:], bankZ[:], AF.Sigmoid, bias=BZ[:])
            # T1 = (hn + b_hh_n) * r
            nc.vector.scalar_tensor_tensor(T1[:], bankN[:], BNH[:], SR[:],
                                           op0=OP.add, op1=OP.mult)
            # T2 = T1 + xn
            nc.vector.tensor_add(T2[:], T1[:], bankX[:])
            # n = tanh(T2 + b_ih_n)
            nc.scalar.activation(NN[:], T2[:], AF.Tanh, bias=BNI[:])
            # h' = n + z*(h - n)
            nc.vector.tensor_sub(U[:], H[:, fd], NN[:])
            nc.gpsimd.tensor_mul(V[:], SZ[:], U[:])
            nc.vector.tensor_add(H[:, fd], NN[:], V[:])

        def body(blk):
            xsb = X[:, blk, :]
            for q in range(UNROLL):
                for g in range(NGROUP):
                    step(q, xsb, g)

        if NBLK == 1:
            body(0)
        else:
            with tc.For_i(0, NBLK, 1,
                          hint_engines=(mybir.EngineType.PE,)) as i:
                body(bass.ds(i, 1))

        # Final FC: out[o, b] = sum_k fc_w[o, k] h[b, k] + fc_b[o]
        for oh in range(2):
            osl = slice(oh * 128, (oh + 1) * 128)
            fc_u = psum.tile([128, HB], f32, tag="bankR0")
            fc_v = psum.tile([128, HB], f32, tag="bankZ0")
            nc.tensor.matmul(fc_u[:], FCW[0:64, osl], H[0:64, :],
                             start=True, stop=True, tile_position=(0, 0))
            nc.tensor.matmul(fc_v[:], FCW[64:128, osl], H[64:128, :],
                             start=True, stop=True, tile_position=(64, 0))
            Ou = work.tile([128, HB], f32, tag="Ou")
            Ov = work.tile([128, HB], f32, tag="Ov")
            nc.scalar.activation(Ou[:], fc_u[:], AF.Identity,
                                 bias=FCB[:, oh:oh + 1])
            nc.scalar.activation(Ov[:], fc_v[:], AF.Identity,
                                 bias=FCB[:, oh:oh + 1])
            nc.gpsimd.dma_start(d_out[osl, 0:HB], Ou[:])
            nc.gpsimd.dma_start(d_out[osl, HB:BC], Ov[:])

    nc.compile()
    return nc


def _host_inputs(x, w_ih, w_hh, b_ih, b_hh, fc_w, fc_b):
    """Build the per-core in_maps (numpy, laid out exactly as SBUF tiles)."""
    f16 = np.float16
    f32 = np.float32
    x = np.asarray(x, f32)
    w_ih = np.asarray(w_ih, f32)
    w_hh = np.asarray(w_hh, f32)
    b_ih = np.asarray(b_ih, f32)
    b_hh = np.asarray(b_hh, f32)
    fc_w = np.asarray(fc_w, f32)
    fc_b = np.asarray(fc_b, f32)

    eye = np.eye(UNROLL, dtype=f32)

    def oh(seg):
        w = w_ih[seg, 0]
        o = np.einsum("pq,m->pqm", eye, w)          # [64, UNROLL, 64]
        return np.concatenate([o, o], 0).astype(f16)  # [128, UNROLL, 64]

    def wstack(seg):
        t = w_hh[seg, :].T                            # [64(k), 64(m)]
        return np.vstack([t, t]).astype(f16)

    def btile(v):
        return np.tile(v.reshape(-1, 1), (2, 1)).astype(f32)  # [128, 1]

    shared = {
        "wr": wstack(slice(0, 64)),
        "wz": wstack(slice(64, 128)),
        "wn": wstack(slice(128, 192)),
        "ohr": oh(slice(0, 64)),
        "ohz": oh(slice(64, 128)),
        "ohn": oh(slice(128, 192)),
        "br": btile(b_ih[0:64] + b_hh[0:64]),
        "bz": btile(b_ih[64:128] + b_hh[64:128]),
        "bnh": btile(b_hh[128:192]),
        "bni": btile(b_ih[128:192]),
        "fcw": np.vstack([fc_w.T, fc_w.T]).astype(f16),  # [128, 256]
        "fcb": np.stack([fc_b[0:128], fc_b[128:256]], 1).astype(f32),
    }

    in_maps = []
    for c in range(NCORES):
        xs = x[c * BC:(c + 1) * BC, :T, 0]            # [BC b, T t]
        xT = np.ascontiguousarray(xs.T)               # [T, BC]
        xr = xT.reshape(NBLK, UNROLL, BC)             # [blk, p, b]
        lo = xr[:, :, 0:HB].transpose(1, 0, 2)        # [64, blk, HB]
        hi = xr[:, :, HB:BC].transpose(1, 0, 2)
        Xh = np.ascontiguousarray(
            np.concatenate([lo, hi], 0)).astype(f16)  # [128, blk, HB]
        m = dict(shared)
        m["xt"] = Xh
        in_maps.append(m)
    return in_maps


def _run(in_maps, trace=False):
    from concourse import bass_utils
    if "nc" not in _CACHE:
        _CACHE["nc"] = _build()
    nc = _CACHE["nc"]
    res = bass_utils.run_bass_kernel_spmd(
        nc, in_maps, core_ids=list(range(NCORES)), trace=trace)
    return res


def kernel(**inputs):
    in_maps = _host_inputs(**inputs)
    res = _run(in_maps, trace=False)
    out = np.empty([B, OUT], np.float32)
    for c in range(NCORES):
        out[c * BC:(c + 1) * BC, :] = res.results[c]["out"].T
    return out


# revision 8
# speedup vs baseline: 1.1154x; 1.1154x over previous
"""GRU decoder kernel for Trainium2 (8 NeuronCores, data-parallel over batch).

Math (PyTorch GRU, gate order r,z,n), per batch element:
    gx_t = x_t * w_ih + b_ih              (input dim == 1 -> rank-1)
    gh_t = h_{t-1} @ w_hh.T + b_hh
    r = sigmoid(gx_r + gh_r); z = sigmoid(gx_z + gh_z)
    n = tanh(gx_n + b_ih_n + r * (gh_n + b_hh_n))
    h_t = (1-z)*n + z*h_{t-1}
    out = h_T @ fc_w.T + fc_b

Device layout (per core, B_c = 1024 batch):
  - partition-stacked: batch 0-511 ("u") on SBUF partitions 0-63,
    batch 512-1023 ("v") on partitions 64-127.  All elementwise ops are
    [128, 512] (gate dim j on partitions within each half, batch on free dim).
  - Two concurrent PE chains via tile_position row/col quadrants:
    u-chain rows 0-1 cols 0-1, v-chain rows 2-3 cols 2-3.
  - x contribution per step via a one-hot K=64 matmul: lhsT[k,m] =
    (k==q)*w_ih[m], rhs = block of 64 timesteps of x -> selects timestep q.
  - biases folded into activation bias (per-partition) and the
    scalar_tensor_tensor per-partition scalar; no bias matmuls.
  - fp16 SBUF tensors, fp32 PSUM accumulation.
"""

import os
import sys

sys.path.insert(0, "/opt/trn_rl_repo")

import numpy as np
from contextlib import ExitStack

HIDDEN = 64
OUT = 256
B = 8192
T = int(os.environ.get("GRU_T", 1024))
NCORES = 8
BC = B // NCORES          # 1024 batch per core
HB = BC // 2              # 512 batch per partition-half
UNROLL = 64               # steps per loop body (one-hot q index is static)
NGROUP = int(os.environ.get("GRU_NGROUP", 2))  # phase-shifted batch groups per core
NBLK = T // UNROLL        # number of 64-step blocks

_CACHE = {}


def _build():
    import concourse.bass as bass
    import concourse.tile as tile
    from concourse import bacc, mybir

    f16 = mybir.dt.float16
    f32 = mybir.dt.float32
    AF = mybir.ActivationFunctionType
    OP = mybir.AluOpType

    nc = bacc.Bacc("TRN2", target_bir_lowering=False, debug=False,
                   num_devices=NCORES)

    d_x = nc.dram_tensor("xt", [128, NBLK, HB], f16, kind="ExternalInput").ap()
    d_wr = nc.dram_tensor("wr", [128, 64], f16, kind="ExternalInput").ap()
    d_wz = nc.dram_tensor("wz", [128, 64], f16, kind="ExternalInput").ap()
    d_wn = nc.dram_tensor("wn", [128, 64], f16, kind="ExternalInput").ap()
    d_ohr = nc.dram_tensor("ohr", [128, UNROLL, 64], f16, kind="ExternalInput").ap()
    d_ohz = nc.dram_tensor("ohz", [128, UNROLL, 64], f16, kind="ExternalInput").ap()
    d_ohn = nc.dram_tensor("ohn", [128, UNROLL, 64], f16, kind="ExternalInput").ap()
    d_br = nc.dram_tensor("br", [128, 1], f32, kind="ExternalInput").ap()
    d_bz = nc.dram_tensor("bz", [128, 1], f32, kind="ExternalInput").ap()
    d_bnh = nc.dram_tensor("bnh", [128, 1], f32, kind="ExternalInput").ap()
    d_bni = nc.dram_tensor("bni", [128, 1], f32, kind="ExternalInput").ap()
    d_fcw = nc.dram_tensor("fcw", [128, OUT], f16, kind="ExternalInput").ap()
    d_fcb = nc.dram_tensor("fcb", [128, 2], f32, kind="ExternalInput").ap()
    d_out = nc.dram_tensor("out", [OUT, BC], f32, kind="ExternalOutput").ap()

    with tile.TileContext(nc) as tc, ExitStack() as ctx:
        singles = ctx.enter_context(tc.tile_pool(name="singles", bufs=1))
        work = ctx.enter_context(tc.tile_pool(name="work", bufs=4))
        psum = ctx.enter_context(tc.tile_pool(name="psum", bufs=1, space="PSUM"))

        X = singles.tile([128, NBLK, HB], f16)
        WR = singles.tile([128, 64], f16)
        WZ = singles.tile([128, 64], f16)
        WN = singles.tile([128, 64], f16)
        OHR = singles.tile([128, UNROLL, 64], f16)
        OHZ = singles.tile([128, UNROLL, 64], f16)
        OHN = singles.tile([128, UNROLL, 64], f16)
        BR = singles.tile([128, 1], f32)
        BZ = singles.tile([128, 1], f32)
        BNH = singles.tile([128, 1], f32)
        BNI = singles.tile([128, 1], f32)
        FCW = singles.tile([128, OUT], f16)
        FCB = singles.tile([128, 2], f32)
        H = singles.tile([128, HB], f16)

        for dst, src in ((X, d_x), (WR, d_wr), (WZ, d_wz), (WN, d_wn),
                         (OHR, d_ohr), (OHZ, d_ohz), (OHN, d_ohn),
                         (BR, d_br), (BZ, d_bz), (BNH, d_bnh), (BNI, d_bni),
                         (FCW, d_fcw), (FCB, d_fcb)):
            nc.gpsimd.dma_start(dst[:], src[:])
        nc.vector.memset(H[:], 0.0)

        HG = HB // NGROUP   # free-dim width per pipelined batch group

        def step(q, xsb, g):
            fd = slice(g * HG, (g + 1) * HG)
            bankR = psum.tile([128, HG], f32, tag=f"bankR{g}")
            bankZ = psum.tile([128, HG], f32, tag=f"bankZ{g}")
            bankN = psum.tile([128, HG], f32, tag=f"bankN{g}")
            bankX = psum.tile([128, HG], f32, tag=f"bankX{g}")
            # x rows live on the OPPOSITE partition half (X-swap) so the
            # one-hot x-matmuls use the other PE row-group: all four
            # quadrant chains (u-h, v-h, u-x, v-x) run concurrently.
            for lo, xlo in ((0, 0), (64, 64)):
                sl = slice(lo, lo + 64)
                xsl = slice(xlo, xlo + 64)
                hs = H[sl, fd]
                xs = xsb[xsl, :, fd]
                tp = (lo, lo)
                xtp = (xlo, lo)
                nc.tensor.matmul(bankR[sl, :], WR[sl, :], hs,
                                 start=True, stop=False, tile_position=tp)
                nc.tensor.matmul(bankR[sl, :], OHR[xsl, q, :], xs,
                                 start=False, stop=True, tile_position=xtp)
                nc.tensor.matmul(bankZ[sl, :], WZ[sl, :], hs,
                                 start=True, stop=False, tile_position=tp)
                nc.tensor.matmul(bankZ[sl, :], OHZ[xsl, q, :], xs,
                                 start=False, stop=True, tile_position=xtp)
                nc.tensor.matmul(bankN[sl, :], WN[sl, :], hs,
                                 start=True, stop=True, tile_position=tp)
                nc.tensor.matmul(bankX[sl, :], OHN[xsl, q, :], xs,
                                 start=True, stop=True, tile_position=xtp)
            SR = work.tile([128, HG], f16, tag=f"SR{g}")
            SZ = work.tile([128, HG], f16, tag=f"SZ{g}")
            T1 = work.tile([128, HG], f16, tag=f"T1{g}")
            T2 = work.tile([128, HG], f16, tag=f"T2{g}")
            NN = work.tile([128, HG], f16, tag=f"NN{g}")
            U = work.tile([128, HG], f16, tag=f"U{g}")
            V = work.tile([128, HG], f16, tag=f"V{g}")
            nc.scalar.activation(SR[:], bankR[:], AF.Sigmoid, bias=BR[:])
            nc.scalar.activation(SZ[:], bankZ[:], AF.Sigmoid, bias=BZ[:])
            # T1 = (hn + b_hh_n) * r
            nc.vector.scalar_tensor_tensor(T1[:], bankN[:], BNH[:], SR[:],
                                           op0=OP.add, op1=OP.mult)
            # T2 = T1 + xn
            nc.vector.tensor_add(T2[:], T1[:], bankX[:])
            # n = tanh(T2 + b_ih_n)
            nc.scalar.activation(NN[:], T2[:], AF.Tanh, bias=BNI[:])
            # h' = n + z*(h - n)
            nc.vector.tensor_sub(U[:], H[:, fd], NN[:])
            nc.vector.tensor_mul(V[:], SZ[:], U[:])
            nc.vector.tensor_add(H[:, fd], NN[:], V[:])

        def body(blk):
            xsb = X[:, blk, :]
            for q in range(UNROLL):
                for g in range(NGROUP):
                    step(q, xsb, g)

        if NBLK == 1:
            body(0)
        else:
            with tc.For_i(0, NBLK, 1,
                          hint_engines=(mybir.EngineType.PE,)) as i:
                body(bass.ds(i, 1))

        # Final FC: out[o, b] = sum_k fc_w[o, k] h[b, k] + fc_b[o]
        for oh in range(2):
            osl = slice(oh * 128, (oh + 1) * 128)
            fc_u = psum.tile([128, HB], f32, tag="bankR0")
            fc_v = psum.tile([128, HB], f32, tag="bankZ0")
            nc.tensor.matmul(fc_u[:], FCW[0:64, osl], H[0:64, :],
                             start=True, stop=True, tile_position=(0, 0))
            nc.tensor.matmul(fc_v[:], FCW[64:128, osl], H[64:128, :],
                             start=True, stop=True, tile_position=(64, 0))
            Ou = work.tile([128, HB], f32, tag="Ou")
            Ov = work.tile([128, HB], f32, tag="Ov")
            nc.scalar.activation(Ou[:], fc_u[:], AF.Identity,
                                 bias=FCB[:, oh:oh + 1])
            nc.scalar.activation(Ov[:], fc_v[:], AF.Identity,
                                 bias=FCB[:, oh:oh + 1])
            nc.gpsimd.dma_start(d_out[osl, 0:HB], Ou[:])
            nc.gpsimd.dma_start(d_out[osl, HB:BC], Ov[:])

    nc.compile()
    return nc


def _host_inputs(x, w_ih, w_hh, b_ih, b_hh, fc_w, fc_b):
    """Build the per-core in_maps (numpy, laid out exactly as SBUF tiles)."""
    f16 = np.float16
    f32 = np.float32
    x = np.asarray(x, f32)
    w_ih = np.asarray(w_ih, f32)
    w_hh = np.asarray(w_hh, f32)
    b_ih = np.asarray(b_ih, f32)
    b_hh = np.asarray(b_hh, f32)
    fc_w = np.asarray(fc_w, f32)
    fc_b = np.asarray(fc_b, f32)

    eye = np.eye(UNROLL, dtype=f32)

    def oh(seg):
        w = w_ih[seg, 0]
        o = np.einsum("pq,m->pqm", eye, w)          # [64, UNROLL, 64]
        return np.concatenate([o, o], 0).astype(f16)  # [128, UNROLL, 64]

    def wstack(seg):
        t = w_hh[seg, :].T                            # [64(k), 64(m)]
        return np.vstack([t, t]).astype(f16)

    def btile(v):
        return np.tile(v.reshape(-1, 1), (2, 1)).astype(f32)  # [128, 1]

    shared = {
        "wr": wstack(slice(0, 64)),
        "wz": wstack(slice(64, 128)),
        "wn": wstack(slice(128, 192)),
        "ohr": oh(slice(0, 64)),
        "ohz": oh(slice(64, 128)),
        "ohn": oh(slice(128, 192)),
        "br": btile(b_ih[0:64] + b_hh[0:64]),
        "bz": btile(b_ih[64:128] + b_hh[64:128]),
        "bnh": btile(b_hh[128:192]),
        "bni": btile(b_ih[128:192]),
        "fcw": np.vstack([fc_w.T, fc_w.T]).astype(f16),  # [128, 256]
        "fcb": np.stack([fc_b[0:128], fc_b[128:256]], 1).astype(f32),
    }

    in_maps = []
    for c in range(NCORES):
        xs = x[c * BC:(c + 1) * BC, :T, 0]            # [BC b, T t]
        xT = np.ascontiguousarray(xs.T)               # [T, BC]
        xr = xT.reshape(NBLK, UNROLL, BC)             # [blk, p, b]
        lo = xr[:, :, 0:HB].transpose(1, 0, 2)        # [64, blk, HB]
        hi = xr[:, :, HB:BC].transpose(1, 0, 2)
        Xh = np.ascontiguousarray(
            np.concatenate([lo, hi], 0)).astype(f16)  # [128, blk, HB]
        m = dict(shared)
        m["xt"] = Xh
        in_maps.append(m)
    return in_maps


def _run(in_maps, trace=False):
    from concourse import bass_utils
    if "nc" not in _CACHE:
        _CACHE["nc"] = _build()
    nc = _CACHE["nc"]
    res = bass_utils.run_bass_kernel_spmd(
        nc, in_maps, core_ids=list(range(NCORES)), trace=trace)
    return res


def kernel(**inputs):
    in_maps = _host_inputs(**inputs)
    res = _run(in_maps, trace=False)
    out = np.empty([B, OUT], np.float32)
    for c in range(NCORES):
        out[c * BC:(c + 1) * BC, :] = res.results[c]["out"].T
    return out
